# revision 12
# baseline (speedup 1.0000x reference)
"""Trainium2 Bass kernel for nn_ContextQueryAttentionLayer.

Math: with B,N,M,D = 32,1024,256,128 the reference's gather index collapses:
  idx[i,j] = (i*M + j) % N = 256*(i%4) + j          (since M=256, N=1024)
so the similarity matrix S (b,n,m) has only 4 distinct rows per batch,
S[b,i,:] = t[b, i%4, :] with t (4,256):
  t[r,j] = q_j.w_q + c_{256r+j}.w_c + sum_d q_{j,d} w_m_d c_{256r+j,d}
Both softmaxes, c2q, sm (a 4x4 matrix per batch) and q2c then collapse to
rank-4-per-batch quantities, leaving a DMA-bound kernel:
  out[b,n] = [ctx_n, C2Q[n%4], ctx_n*C2Q[n%4], ctx_n*Q2C[n%4]]

Sharding: data-parallel over batch, 4 batches per core on 8 cores.
On-core layout: rows n=128k+p -> partition p (so n%4 == p%4).

v5: HW showed DVE/Pool elementwise ops cost ~1us nearly independent of
size, so the schedule minimizes big-engine op count: ONE mega output tile
[128,BPC,8,512] (ctx DMA-loaded into cols 0:128; one 2KB-descriptor
output DMA per batch), batched query prep, per-GROUP t-columns (2 muls +
2 reduces per 2-batch group), softmax normalizations folded into the
PSUM-copy scales, C2Q broadcast written by an SBUF->SBUF DMA (queues are
idle mid-kernel), products alternate DVE/GpSimd as full-size ops. DMA
triggers spread over the sync+scalar sequencers (descriptor generation
serializes per engine); issue order tracks expected readiness.
"""

import numpy as np

B, N, M, D = 32, 1024, 256, 128
NCORES = 8
BPC = B // NCORES  # batches per core

_prog = None

# packed constant layout: name -> (partitions, col_start, col_len)
_CST_COLS = {
    "ident": (128, 0, 128),
    "wmb": (128, 128, 128),
    "wcb": (128, 256, 128),
    "wqb": (128, 384, 128),
    "b4": (4, 512, 128),
    "i16": (16, 640, 16),
    "pairsel": (16, 656, 8),
    "hsel": (16, 680, 4),
    "rsel": (128, 684, 4),
}
_CST_W = 688


def _build_program():
    import concourse.bacc as bacc
    import concourse.mybir as mybir
    from concourse.tile import TileContext

    fp32 = mybir.dt.float32
    nc = bacc.Bacc("TRN2", target_bir_lowering=False, name="cqattn")

    ctx_d = nc.dram_tensor("ctx", [BPC, N, D], fp32, kind="ExternalInput")
    qry_d = nc.dram_tensor("qry", [BPC, M, D], fp32, kind="ExternalInput")
    cstp_d = nc.dram_tensor("cstp", [128, _CST_W], fp32, kind="ExternalInput")
    out_d = nc.dram_tensor("out", [BPC, N, 4 * D], fp32, kind="ExternalOutput")

    Exp = mybir.ActivationFunctionType.Exp
    Copy = mybir.ActivationFunctionType.Copy
    add = mybir.AluOpType.add
    X = mybir.AxisListType.X

    with TileContext(nc) as tc:
        with (
            tc.tile_pool(name="consts", bufs=1) as consts,
            tc.tile_pool(name="io", bufs=1) as io,
            tc.tile_pool(name="work", bufs=2) as work,
            tc.tile_pool(name="small", bufs=2) as small,
            tc.tile_pool(name="outp", bufs=1) as outp,
            tc.tile_pool(name="ps_tr", bufs=2, space="PSUM") as ps_tr,
            tc.tile_pool(name="ps_sm", bufs=1, space="PSUM") as ps_sm,
            tc.tile_pool(name="ps_mm", bufs=2, space="PSUM") as ps_mm,
            tc.tile_pool(name="ps_cs", bufs=1, space="PSUM") as ps_cs,
            tc.tile_pool(name="ps_rep", bufs=2, space="PSUM") as ps_rep,
        ):
            qry_mega = io.tile([128, BPC, 2, 128], fp32, tag="qry", name="qry_mega")
            out_mega = outp.tile(
                [128, BPC, 8, 512], fp32, tag="out", name="out_mega"
            )

            # ---- loads. Descriptor generation serializes per issuing
            # engine: ctx 0/1 expand on the (early-idle) Act sequencer.
            def load_ctx(b, eng):
                for k0 in (0, 4):
                    eng.dma_start(
                        out=out_mega[:, b, k0 : k0 + 4, 0:128],
                        in_=ctx_d[b, 128 * k0 : 128 * (k0 + 4)].rearrange(
                            "(k p) d -> p k d", p=128
                        ),
                    )

            load_ctx(0, nc.scalar)
            load_ctx(1, nc.scalar)
            cstp = consts.tile([128, _CST_W], fp32, tag="cstp", name="cstp")
            nc.sync.dma_start(out=cstp, in_=cstp_d[...])
            cst = {
                n: cstp[:p, c0 : c0 + cl] for n, (p, c0, cl) in _CST_COLS.items()
            }
            for b in range(BPC):
                nc.sync.dma_start(
                    out=qry_mega[:, b],
                    in_=qry_d[b].rearrange("(h p) d -> p h d", p=128),
                )
            load_ctx(2, nc.sync)
            load_ctx(3, nc.sync)

            # ---- batched query prep: qwcT = qry*w_m + w_c, sq = qry . w_q
            qwcT = work.tile([128, BPC, 2, 128], fp32, tag="qwcT")
            nc.vector.tensor_mul(
                qwcT,
                qry_mega,
                cst["wmb"]
                .rearrange("p (u v d) -> p u v d", u=1, v=1)
                .to_broadcast([128, BPC, 2, 128]),
            )
            nc.vector.tensor_add(
                qwcT,
                qwcT,
                cst["wcb"]
                .rearrange("p (u v d) -> p u v d", u=1, v=1)
                .to_broadcast([128, BPC, 2, 128]),
            )
            sq_tmp = work.tile([128, BPC, 2, 128], fp32, tag="sq_tmp")
            nc.gpsimd.tensor_mul(
                sq_tmp,
                qry_mega,
                cst["wqb"]
                .rearrange("p (u v d) -> p u v d", u=1, v=1)
                .to_broadcast([128, BPC, 2, 128]),
            )
            sq_col = small.tile([128, BPC, 2], fp32, tag="sq_col")
            nc.vector.tensor_reduce(out=sq_col, in_=sq_tmp, axis=X, op=add)

            cs_sb = [
                small.tile([4, 128], fp32, tag=f"cs{b}", name=f"cs{b}")
                for b in range(BPC)
            ]
            t_g = [
                small.tile([128, 16], fp32, tag=f"t_g{g}", name=f"t_g{g}")
                for g in range(2)
            ]

            def batch_cs_mm(b):
                """n%4 column sums, TensorE part (PE + returns psum tile)."""
                ctx_b = out_mega[:, b, :, 0:128]
                cs_ps = ps_cs.tile([4, 4, 128], fp32, tag="cs")
                nc.tensor.matmul(
                    cs_ps, cst["rsel"], ctx_b[:, 0:4, :], start=True, stop=False
                )
                nc.tensor.matmul(
                    cs_ps, cst["rsel"], ctx_b[:, 4:8, :], start=False, stop=True
                )
                return cs_ps

            def cs_fold(b, cs_ps):
                nc.vector.tensor_reduce(
                    out=cs_sb[b],
                    in_=cs_ps.rearrange("p k d -> p d k"),
                    axis=X,
                    op=add,
                )

            def group_t(g):
                """t columns for batches 2g,2g+1 then s_q add:
                t_g[p, 8b'+2r+h] = t[b, r, 128h+p]."""
                ctx_gv = out_mega[:, 2 * g : 2 * g + 2, :, 0:128].rearrange(
                    "p b (r h) d -> p h b r d", h=2
                )
                t_v = t_g[g].rearrange("p (b r h) -> p h b r", b=2, r=4, h=2)
                for h in range(2):
                    g_tmp = work.tile([128, 2, 4, 128], fp32, tag="g_tmp")
                    nc.vector.tensor_mul(
                        g_tmp,
                        ctx_gv[:, h],
                        qwcT[:, 2 * g : 2 * g + 2, h, :]
                        .rearrange("p b (u d) -> p b u d", u=1)
                        .to_broadcast([128, 2, 4, 128]),
                    )
                    nc.vector.tensor_reduce(
                        out=t_v[:, h], in_=g_tmp, axis=X, op=add
                    )
                nc.gpsimd.tensor_add(
                    t_g[g].rearrange("p (b r h) -> p b r h", b=2, h=2),
                    t_g[g].rearrange("p (b r h) -> p b r h", b=2, h=2),
                    sq_col[:, 2 * g : 2 * g + 2].rearrange(
                        "p b (u h) -> p b u h", u=1
                    ).to_broadcast([128, 2, 4, 2]),
                )

            # per-group softmax state carried to the batch tails
            eT_sb = [None, None]
            sqT2 = [None, None]
            rec4 = [[None, None], [None, None]]   # 1/S_r       per (g, b')
            rec4q = [[None, None], [None, None]]  # 1/(256 S_r) per (g, b')

            def group_softmax(g):
                """Softmax on raw exp; 1/S_r and 1/(256 S_r) ride the
                c2q/q2c PSUM copies, 1/U_q rides sqT."""
                t16_ps = ps_tr.tile([16, 128], fp32, tag="tr")
                nc.tensor.transpose(t16_ps, t_g[g], cst["ident"])
                e16 = small.tile([16, 128], fp32, tag=f"e16_{g}", name=f"e16_{g}")
                rowsumc = small.tile([16, 1], fp32, tag="rowsumc")
                nc.scalar.activation(
                    out=e16, in_=t16_ps, func=Exp, accum_out=rowsumc
                )
                for b2 in range(2):
                    pairs_ps = ps_sm.tile([4, 1], fp32, tag="sm")
                    nc.tensor.matmul(
                        pairs_ps,
                        cst["pairsel"][:, 4 * b2 : 4 * b2 + 4],
                        rowsumc,
                        start=True,
                        stop=True,
                    )
                    rec4[g][b2] = small.tile(
                        [4, 1], fp32, tag=f"rec4_{g}{b2}", name=f"rec4_{g}{b2}"
                    )
                    nc.vector.reciprocal(out=rec4[g][b2], in_=pairs_ps)
                    rec4q[g][b2] = small.tile(
                        [4, 1], fp32, tag=f"rec4q_{g}{b2}", name=f"rec4q_{g}{b2}"
                    )
                    nc.vector.tensor_scalar_mul(
                        rec4q[g][b2], rec4[g][b2], 1.0 / 256.0
                    )
                u2_ps = ps_sm.tile([4, 128], fp32, tag="sm")
                nc.tensor.matmul(u2_ps, cst["hsel"], e16, start=True, stop=True)
                u2 = small.tile([4, 128], fp32, tag="u2")
                nc.scalar.copy(out=u2, in_=u2_ps)

                eT_ps = ps_tr.tile([128, 16], fp32, tag="tr")
                nc.tensor.transpose(eT_ps, e16, cst["i16"])
                eT_sb[g] = small.tile(
                    [128, 16], fp32, tag=f"eT_{g}", name=f"eT_{g}"
                )
                nc.vector.tensor_copy(out=eT_sb[g], in_=eT_ps)
                u2T_ps = ps_tr.tile([128, 4], fp32, tag="tr")
                nc.tensor.transpose(u2T_ps, u2, cst["i16"][:4, :4])
                recu = small.tile([128, 4], fp32, tag="recu")
                nc.vector.reciprocal(out=recu, in_=u2T_ps)
                sqT2[g] = small.tile(
                    [128, 16], fp32, tag=f"sqT_{g}", name=f"sqT_{g}"
                )
                nc.vector.tensor_mul(
                    sqT2[g].rearrange("p (b r h) -> p b r h", b=2, h=2),
                    eT_ps.rearrange("p (b r h) -> p b r h", b=2, h=2),
                    recu.rearrange("p (b u h) -> p b u h", u=1, h=2)
                    .to_broadcast([128, 2, 4, 2]),
                )

            def batch_tail(b):
                g, b2 = b // 2, b % 2
                q0 = 8 * b2
                eT = eT_sb[g][:, q0 : q0 + 8].rearrange("p (r h) -> p r h", r=4)
                sqT = sqT2[g][:, q0 : q0 + 8].rearrange("p (r h) -> p r h", r=4)

                # raw Gram matrix: sm4t_ps[a,c] = sum_q e[a,q] e[c,q] / U_q
                sm4t_ps = ps_mm.tile([4, 4], fp32, tag="mm")
                for h in range(2):
                    nc.tensor.matmul(
                        sm4t_ps, sqT[:, :, h], eT[:, :, h],
                        start=(h == 0), stop=(h == 1),
                    )
                sm4t = small.tile([4, 4], fp32, tag="sm4t")
                nc.vector.tensor_copy(out=sm4t, in_=sm4t_ps)

                # C2Q[r,d] = (1/S_r) sum_q e[r,q] qry[q,d]
                c2q_ps = ps_mm.tile([4, 128], fp32, tag="mm")
                for h in range(2):
                    nc.tensor.matmul(
                        c2q_ps, eT[:, :, h], qry_mega[:, b, h, :],
                        start=(h == 0), stop=(h == 1),
                    )
                c2q = small.tile([4, 128], fp32, tag="c2q")
                nc.scalar.activation(
                    out=c2q, in_=c2q_ps, func=Copy, scale=rec4[g][b2]
                )

                # Q2C[r,d] = (1/(256 S_r)) sum_{r'} sm4t[r',r] CS[r',d]
                q2c_ps = ps_mm.tile([4, 128], fp32, tag="mm")
                nc.tensor.matmul(q2c_ps, sm4t, cs_sb[b], start=True, stop=True)
                q2c = small.tile([4, 128], fp32, tag="q2c")
                nc.scalar.activation(
                    out=q2c, in_=q2c_ps, func=Copy, scale=rec4q[g][b2]
                )

                # broadcast rows r -> 128 partitions (p%4 pattern)
                repc_ps = ps_rep.tile([128, 128], fp32, tag="rep")
                nc.tensor.matmul(repc_ps, cst["b4"], c2q, start=True, stop=True)
                repq_ps = ps_rep.tile([128, 128], fp32, tag="rep")
                nc.tensor.matmul(repq_ps, cst["b4"], q2c, start=True, stop=True)
                repc_sb = small.tile([128, 128], fp32, tag="repc_sb")
                nc.scalar.copy(out=repc_sb, in_=repc_ps)
                repq_sb = small.tile([128, 128], fp32, tag="repq_sb")
                nc.scalar.copy(out=repq_sb, in_=repq_ps)

                # C2Q columns via SBUF->SBUF broadcast DMA (queues are idle
                # mid-kernel); products alternate DVE (PSUM src) / GpSimd
                # (SBUF src) so each batch uses both engines in parallel.
                ctx_b = out_mega[:, b, :, 0:128]
                nc.sync.dma_start(
                    out=out_mega[:, b, :, 128:256],
                    in_=repc_sb.rearrange("p (u d) -> p u d", u=1)
                    .to_broadcast([128, 8, 128]),
                )
                dve_c = b % 2 == 0
                prod_c_eng = nc.vector if dve_c else nc.gpsimd
                prod_c_src = repc_ps if dve_c else repc_sb
                prod_d_eng = nc.gpsimd if dve_c else nc.vector
                prod_d_src = repq_sb if dve_c else repq_ps
                prod_c_eng.tensor_mul(
                    out_mega[:, b, :, 256:384],
                    ctx_b,
                    prod_c_src.rearrange("p (u d) -> p u d", u=1)
                    .to_broadcast([128, 8, 128]),
                )
                prod_d_eng.tensor_mul(
                    out_mega[:, b, :, 384:512],
                    ctx_b,
                    prod_d_src.rearrange("p (u d) -> p u d", u=1)
                    .to_broadcast([128, 8, 128]),
                )
                nc.sync.dma_start(
                    out=out_d[b].rearrange("(k p) c -> p k c", p=128),
                    in_=out_mega[:, b],
                )

            # issue order ~= expected readiness order
            cs0 = batch_cs_mm(0)
            cs_fold(0, cs0)
            cs1 = batch_cs_mm(1)
            cs_fold(1, cs1)
            group_t(0)
            group_softmax(0)
            cs2 = batch_cs_mm(2)
            cs3 = batch_cs_mm(3)
            group_t(1)
            batch_tail(0)
            batch_tail(1)
            group_softmax(1)
            cs_fold(2, cs2)
            cs_fold(3, cs3)
            batch_tail(2)
            batch_tail(3)
    nc.compile()
    return nc


def _get_program():
    global _prog
    if _prog is None:
        _prog = _build_program()
    return _prog


def _make_const_inputs(w):
    w = np.ascontiguousarray(w, dtype=np.float32)
    w_q, w_c, w_m = w[:D, 0], w[D : 2 * D, 0], w[2 * D :, 0]
    p = np.arange(128)
    q = np.arange(16)
    # within a 2-batch group: q = 8b' + 2r + h; pair j = 4b' + r; u k = 2b' + h
    pairsel = (
        (q[:, None] // 8 == np.arange(8)[None, :] // 4)
        & ((q[:, None] % 8) // 2 == np.arange(8)[None, :] % 4)
    ).astype(np.float32)
    hsel = (
        (q[:, None] // 8 == np.arange(4)[None, :] // 2)
        & (q[:, None] % 2 == np.arange(4)[None, :] % 2)
    ).astype(np.float32)
    vals = {
        "ident": np.eye(128, dtype=np.float32),
        "i16": np.eye(16, dtype=np.float32),
        "wmb": np.broadcast_to(w_m[None, :], (128, 128)),
        "wcb": np.broadcast_to(w_c[None, :], (128, 128)),
        "wqb": np.broadcast_to(w_q[None, :], (128, 128)),
        "pairsel": pairsel,
        "hsel": hsel,
        "rsel": (p[:, None] % 4 == np.arange(4)[None, :]).astype(np.float32),
        "b4": (np.arange(4)[:, None] == p[None, :] % 4).astype(np.float32),
    }
    packed = np.zeros((128, _CST_W), dtype=np.float32)
    for n, (parts, c0, cl) in _CST_COLS.items():
        packed[:parts, c0 : c0 + cl] = vals[n]
    return {"cstp": packed}


def _run(context, query, w, trace=False):
    from concourse.bass_utils import run_bass_kernel_spmd

    nc = _get_program()
    context = np.ascontiguousarray(context, dtype=np.float32)
    query = np.ascontiguousarray(query, dtype=np.float32)
    consts = _make_const_inputs(w)

    in_maps = []
    for c in range(NCORES):
        m = {
            "ctx": context[c * BPC : (c + 1) * BPC],
            "qry": query[c * BPC : (c + 1) * BPC],
        }
        m.update(consts)
        in_maps.append(m)

    res = run_bass_kernel_spmd(
        nc, in_maps, core_ids=list(range(NCORES)), trace=trace
    )
    out = np.concatenate([res.results[c]["out"] for c in range(NCORES)], axis=0)
    return out, res


def kernel(context, query, c_mask, q_mask, w):
    out, _ = _run(context, query, w, trace=False)
    return out


# revision 13
# speedup vs baseline: 1.1390x; 1.1390x over previous
"""Trainium2 Bass kernel for nn_ContextQueryAttentionLayer.

Math: with B,N,M,D = 32,1024,256,128 the reference's gather index collapses:
  idx[i,j] = (i*M + j) % N = 256*(i%4) + j          (since M=256, N=1024)
so the similarity matrix S (b,n,m) has only 4 distinct rows per batch,
S[b,i,:] = t[b, i%4, :] with t (4,256):
  t[r,j] = q_j.w_q + c_{256r+j}.w_c + sum_d q_{j,d} w_m_d c_{256r+j,d}
Both softmaxes, c2q, sm (a 4x4 matrix per batch) and q2c then collapse to
rank-4-per-batch quantities, leaving a DMA-bound kernel:
  out[b,n] = [ctx_n, C2Q[n%4], ctx_n*C2Q[n%4], ctx_n*Q2C[n%4]]

Sharding: data-parallel over batch, 4 batches per core on 8 cores.
On-core layout: rows n=128k+p -> partition p (so n%4 == p%4).

v6: HW showed elementwise ops cost ~1-3us nearly independent of size, so
the schedule minimizes DVE/Pool op count and keeps the softmax chain off
those engines: ONE mega output tile [128,BPC,8,512] (ctx DMA-loaded into
cols 0:128; one 2KB-descriptor output DMA per batch), batched query prep,
per-GROUP t-columns (2 muls + 2 reduces per 2-batch group), the s_q bias
ACCUMULATED into the transpose matmul on PE (start/stop chaining) instead
of a Pool add, softmax normalizations folded into the PSUM-copy scales,
products alternate DVE/GpSimd as full-size ops. DMA triggers spread over
sync+scalar sequencers (descriptor expansion ~4.8ns/BD serializes per
engine); issue order tracks expected readiness (engines run in order).
"""

import numpy as np

B, N, M, D = 32, 1024, 256, 128
NCORES = 8
BPC = B // NCORES  # batches per core

_prog = None

# packed constant layout: name -> (partitions, col_start, col_len)
_CST_COLS = {
    "ident": (128, 0, 128),
    "wmb": (128, 128, 128),
    "wcb": (128, 256, 128),
    "wqb": (128, 384, 128),
    "b4": (4, 512, 128),
    "i16": (16, 640, 16),
    "pairsel": (16, 656, 8),
    "hsel": (16, 680, 4),
    "rsel": (128, 684, 4),
    "sqsel0": (8, 688, 16),
    "sqsel1": (8, 704, 16),
}
_CST_W = 720


def _build_program():
    import concourse.bacc as bacc
    import concourse.mybir as mybir
    from concourse.tile import TileContext

    fp32 = mybir.dt.float32
    nc = bacc.Bacc("TRN2", target_bir_lowering=False, name="cqattn")

    ctx_d = nc.dram_tensor("ctx", [BPC, N, D], fp32, kind="ExternalInput")
    qry_d = nc.dram_tensor("qry", [BPC, M, D], fp32, kind="ExternalInput")
    cstp_d = nc.dram_tensor("cstp", [128, _CST_W], fp32, kind="ExternalInput")
    out_d = nc.dram_tensor("out", [BPC, N, 4 * D], fp32, kind="ExternalOutput")

    Exp = mybir.ActivationFunctionType.Exp
    Copy = mybir.ActivationFunctionType.Copy
    add = mybir.AluOpType.add
    X = mybir.AxisListType.X

    with TileContext(nc) as tc:
        with (
            tc.tile_pool(name="consts", bufs=1) as consts,
            tc.tile_pool(name="io", bufs=1) as io,
            tc.tile_pool(name="work", bufs=2) as work,
            tc.tile_pool(name="small", bufs=2) as small,
            tc.tile_pool(name="outp", bufs=1) as outp,
            tc.tile_pool(name="ps_tr", bufs=2, space="PSUM") as ps_tr,
            tc.tile_pool(name="ps_sm", bufs=1, space="PSUM") as ps_sm,
            tc.tile_pool(name="ps_mm", bufs=2, space="PSUM") as ps_mm,
            tc.tile_pool(name="ps_cs", bufs=1, space="PSUM") as ps_cs,
            tc.tile_pool(name="ps_rep", bufs=2, space="PSUM") as ps_rep,
        ):
            qry_mega = io.tile([128, BPC, 2, 128], fp32, tag="qry", name="qry_mega")
            out_mega = outp.tile(
                [128, BPC, 8, 512], fp32, tag="out", name="out_mega"
            )

            # ---- loads. Descriptor expansion serializes per issuing
            # engine; balance: qry (gates prep) + ctx b1 expand on the
            # early-idle Act sequencer, ctx b0/b2/b3 on sync.
            def load_ctx(b, eng):
                for k0 in (0, 4):
                    eng.dma_start(
                        out=out_mega[:, b, k0 : k0 + 4, 0:128],
                        in_=ctx_d[b, 128 * k0 : 128 * (k0 + 4)].rearrange(
                            "(k p) d -> p k d", p=128
                        ),
                    )

            cstp = consts.tile([128, _CST_W], fp32, tag="cstp", name="cstp")
            nc.scalar.dma_start(out=cstp, in_=cstp_d[...])
            cst = {
                n: cstp[:p, c0 : c0 + cl] for n, (p, c0, cl) in _CST_COLS.items()
            }
            for b in range(BPC):
                nc.scalar.dma_start(
                    out=qry_mega[:, b],
                    in_=qry_d[b].rearrange("(h p) d -> p h d", p=128),
                )
            load_ctx(0, nc.sync)
            load_ctx(1, nc.scalar)
            load_ctx(2, nc.sync)
            load_ctx(3, nc.sync)

            # ---- batched query prep: qwcT = qry*w_m + w_c, sq = qry . w_q
            qwcT = work.tile([128, BPC, 2, 128], fp32, tag="qwcT")
            nc.vector.tensor_mul(
                qwcT,
                qry_mega,
                cst["wmb"]
                .rearrange("p (u v d) -> p u v d", u=1, v=1)
                .to_broadcast([128, BPC, 2, 128]),
            )
            nc.vector.tensor_add(
                qwcT,
                qwcT,
                cst["wcb"]
                .rearrange("p (u v d) -> p u v d", u=1, v=1)
                .to_broadcast([128, BPC, 2, 128]),
            )
            sq_tmp = work.tile([128, BPC, 2, 128], fp32, tag="sq_tmp")
            nc.gpsimd.tensor_mul(
                sq_tmp,
                qry_mega,
                cst["wqb"]
                .rearrange("p (u v d) -> p u v d", u=1, v=1)
                .to_broadcast([128, BPC, 2, 128]),
            )
            sq_col = small.tile([128, BPC, 2], fp32, tag="sq_col")
            nc.vector.tensor_reduce(out=sq_col, in_=sq_tmp, axis=X, op=add)
            # transposed s_q pattern: sq8[2b+h, p] = s_q[b, 128h+p]
            sq8_ps = ps_sm.tile([8, 128], fp32, tag="sm")
            nc.tensor.matmul(
                sq8_ps, sq_col.rearrange("p b h -> p (b h)"), cst["ident"],
                start=True, stop=True,
            )
            sq8 = small.tile([8, 128], fp32, tag="sq8")
            nc.scalar.copy(out=sq8, in_=sq8_ps)

            cs_sb = [
                small.tile([4, 128], fp32, tag=f"cs{b}", name=f"cs{b}")
                for b in range(BPC)
            ]
            t_g = [
                small.tile([128, 16], fp32, tag=f"t_g{g}", name=f"t_g{g}")
                for g in range(2)
            ]

            def batch_cs_mm(b):
                """n%4 column sums, TensorE part (returns psum tile)."""
                ctx_b = out_mega[:, b, :, 0:128]
                cs_ps = ps_cs.tile([4, 4, 128], fp32, tag="cs")
                nc.tensor.matmul(
                    cs_ps, cst["rsel"], ctx_b[:, 0:4, :], start=True, stop=False
                )
                nc.tensor.matmul(
                    cs_ps, cst["rsel"], ctx_b[:, 4:8, :], start=False, stop=True
                )
                return cs_ps

            def cs_fold(b, cs_ps):
                nc.vector.tensor_reduce(
                    out=cs_sb[b],
                    in_=cs_ps.rearrange("p k d -> p d k"),
                    axis=X,
                    op=add,
                )

            def group_t(g):
                """t columns (sans s_q) for batches 2g,2g+1:
                t_g[p, 8b'+2r+h] = t[b, r, 128h+p]."""
                ctx_gv = out_mega[:, 2 * g : 2 * g + 2, :, 0:128].rearrange(
                    "p b (r h) d -> p h b r d", h=2
                )
                t_v = t_g[g].rearrange("p (b r h) -> p h b r", b=2, r=4, h=2)
                for h in range(2):
                    g_tmp = work.tile([128, 2, 4, 128], fp32, tag="g_tmp")
                    nc.vector.tensor_mul(
                        g_tmp,
                        ctx_gv[:, h],
                        qwcT[:, 2 * g : 2 * g + 2, h, :]
                        .rearrange("p b (u d) -> p b u d", u=1)
                        .to_broadcast([128, 2, 4, 128]),
                    )
                    nc.vector.tensor_reduce(
                        out=t_v[:, h], in_=g_tmp, axis=X, op=add
                    )

            # per-group softmax state carried to the batch tails
            eT_sb = [None, None]
            sqT2 = [None, None]
            rec4 = [[None, None], [None, None]]   # 1/S_r       per (g, b')
            rec4q = [[None, None], [None, None]]  # 1/(256 S_r) per (g, b')

            def group_softmax(g):
                """Softmax on raw exp; s_q rides the transpose matmul as an
                accumulated bias; 1/S_r and 1/(256 S_r) ride the c2q/q2c
                PSUM copies; 1/U_q rides sqT."""
                t16_ps = ps_tr.tile([16, 128], fp32, tag="tr")
                nc.tensor.matmul(
                    t16_ps, t_g[g], cst["ident"], start=True, stop=False
                )
                nc.tensor.matmul(
                    t16_ps, cst[f"sqsel{g}"], sq8, start=False, stop=True
                )
                e16 = small.tile([16, 128], fp32, tag=f"e16_{g}", name=f"e16_{g}")
                rowsumc = small.tile([16, 1], fp32, tag="rowsumc")
                nc.scalar.activation(
                    out=e16, in_=t16_ps, func=Exp, accum_out=rowsumc
                )
                for b2 in range(2):
                    pairs_ps = ps_sm.tile([4, 1], fp32, tag="sm")
                    nc.tensor.matmul(
                        pairs_ps,
                        cst["pairsel"][:, 4 * b2 : 4 * b2 + 4],
                        rowsumc,
                        start=True,
                        stop=True,
                    )
                    rec4[g][b2] = small.tile(
                        [4, 1], fp32, tag=f"rec4_{g}{b2}", name=f"rec4_{g}{b2}"
                    )
                    nc.vector.reciprocal(out=rec4[g][b2], in_=pairs_ps)
                    rec4q[g][b2] = small.tile(
                        [4, 1], fp32, tag=f"rec4q_{g}{b2}", name=f"rec4q_{g}{b2}"
                    )
                    nc.vector.tensor_scalar_mul(
                        rec4q[g][b2], rec4[g][b2], 1.0 / 256.0
                    )
                u2_ps = ps_sm.tile([4, 128], fp32, tag="sm")
                nc.tensor.matmul(u2_ps, cst["hsel"], e16, start=True, stop=True)
                u2 = small.tile([4, 128], fp32, tag="u2")
                nc.scalar.copy(out=u2, in_=u2_ps)

                eT_ps = ps_tr.tile([128, 16], fp32, tag="tr")
                nc.tensor.transpose(eT_ps, e16, cst["i16"])
                eT_sb[g] = small.tile(
                    [128, 16], fp32, tag=f"eT_{g}", name=f"eT_{g}"
                )
                nc.vector.tensor_copy(out=eT_sb[g], in_=eT_ps)
                u2T_ps = ps_tr.tile([128, 4], fp32, tag="tr")
                nc.tensor.transpose(u2T_ps, u2, cst["i16"][:4, :4])
                recu = small.tile([128, 4], fp32, tag="recu")
                nc.vector.reciprocal(out=recu, in_=u2T_ps)
                sqT2[g] = small.tile(
                    [128, 16], fp32, tag=f"sqT_{g}", name=f"sqT_{g}"
                )
                nc.vector.tensor_mul(
                    sqT2[g].rearrange("p (b r h) -> p b r h", b=2, h=2),
                    eT_ps.rearrange("p (b r h) -> p b r h", b=2, h=2),
                    recu.rearrange("p (b u h) -> p b u h", u=1, h=2)
                    .to_broadcast([128, 2, 4, 2]),
                )

            def batch_tail(b):
                g, b2 = b // 2, b % 2
                q0 = 8 * b2
                eT = eT_sb[g][:, q0 : q0 + 8].rearrange("p (r h) -> p r h", r=4)
                sqT = sqT2[g][:, q0 : q0 + 8].rearrange("p (r h) -> p r h", r=4)

                # raw Gram matrix: sm4t_ps[a,c] = sum_q e[a,q] e[c,q] / U_q
                sm4t_ps = ps_mm.tile([4, 4], fp32, tag="mm")
                for h in range(2):
                    nc.tensor.matmul(
                        sm4t_ps, sqT[:, :, h], eT[:, :, h],
                        start=(h == 0), stop=(h == 1),
                    )
                sm4t = small.tile([4, 4], fp32, tag="sm4t")
                nc.vector.tensor_copy(out=sm4t, in_=sm4t_ps)

                # C2Q[r,d] = (1/S_r) sum_q e[r,q] qry[q,d]
                c2q_ps = ps_mm.tile([4, 128], fp32, tag="mm")
                for h in range(2):
                    nc.tensor.matmul(
                        c2q_ps, eT[:, :, h], qry_mega[:, b, h, :],
                        start=(h == 0), stop=(h == 1),
                    )
                c2q = small.tile([4, 128], fp32, tag="c2q")
                nc.scalar.activation(
                    out=c2q, in_=c2q_ps, func=Copy, scale=rec4[g][b2]
                )

                # Q2C[r,d] = (1/(256 S_r)) sum_{r'} sm4t[r',r] CS[r',d]
                q2c_ps = ps_mm.tile([4, 128], fp32, tag="mm")
                nc.tensor.matmul(q2c_ps, sm4t, cs_sb[b], start=True, stop=True)
                q2c = small.tile([4, 128], fp32, tag="q2c")
                nc.scalar.activation(
                    out=q2c, in_=q2c_ps, func=Copy, scale=rec4q[g][b2]
                )

                # broadcast rows r -> 128 partitions (p%4 pattern)
                repc_ps = ps_rep.tile([128, 128], fp32, tag="rep")
                nc.tensor.matmul(repc_ps, cst["b4"], c2q, start=True, stop=True)
                repq_ps = ps_rep.tile([128, 128], fp32, tag="rep")
                nc.tensor.matmul(repq_ps, cst["b4"], q2c, start=True, stop=True)
                repc_sb = small.tile([128, 128], fp32, tag="repc_sb")
                nc.scalar.copy(out=repc_sb, in_=repc_ps)
                repq_sb = small.tile([128, 128], fp32, tag="repq_sb")
                nc.scalar.copy(out=repq_sb, in_=repq_ps)

                # C2Q columns (Act copy); products alternate DVE (PSUM src)
                # / GpSimd (SBUF src) so each batch uses both in parallel.
                ctx_b = out_mega[:, b, :, 0:128]
                nc.scalar.copy(
                    out=out_mega[:, b, :, 128:256],
                    in_=repc_ps.rearrange("p (u d) -> p u d", u=1)
                    .to_broadcast([128, 8, 128]),
                )
                dve_c = b % 2 == 0
                prod_c_eng = nc.vector if dve_c else nc.gpsimd
                prod_c_src = repc_ps if dve_c else repc_sb
                prod_d_eng = nc.gpsimd if dve_c else nc.vector
                prod_d_src = repq_sb if dve_c else repq_ps
                prod_c_eng.tensor_mul(
                    out_mega[:, b, :, 256:384],
                    ctx_b,
                    prod_c_src.rearrange("p (u d) -> p u d", u=1)
                    .to_broadcast([128, 8, 128]),
                )
                prod_d_eng.tensor_mul(
                    out_mega[:, b, :, 384:512],
                    ctx_b,
                    prod_d_src.rearrange("p (u d) -> p u d", u=1)
                    .to_broadcast([128, 8, 128]),
                )
                nc.sync.dma_start(
                    out=out_d[b].rearrange("(k p) c -> p k c", p=128),
                    in_=out_mega[:, b],
                )

            # issue order ~= expected readiness order
            cs0 = batch_cs_mm(0)
            cs_fold(0, cs0)
            cs1 = batch_cs_mm(1)
            cs_fold(1, cs1)
            group_t(0)
            group_softmax(0)
            cs2 = batch_cs_mm(2)
            cs3 = batch_cs_mm(3)
            group_t(1)
            batch_tail(0)
            batch_tail(1)
            group_softmax(1)
            cs_fold(2, cs2)
            cs_fold(3, cs3)
            batch_tail(2)
            batch_tail(3)
    nc.compile()
    return nc


def _get_program():
    global _prog
    if _prog is None:
        _prog = _build_program()
    return _prog


def _make_const_inputs(w):
    w = np.ascontiguousarray(w, dtype=np.float32)
    w_q, w_c, w_m = w[:D, 0], w[D : 2 * D, 0], w[2 * D :, 0]
    p = np.arange(128)
    q = np.arange(16)
    # within a 2-batch group: q = 8b' + 2r + h; pair j = 4b' + r; u k = 2b' + h
    pairsel = (
        (q[:, None] // 8 == np.arange(8)[None, :] // 4)
        & ((q[:, None] % 8) // 2 == np.arange(8)[None, :] % 4)
    ).astype(np.float32)
    hsel = (
        (q[:, None] // 8 == np.arange(4)[None, :] // 2)
        & (q[:, None] % 2 == np.arange(4)[None, :] % 2)
    ).astype(np.float32)
    # sqsel_g[2b+h, 8b'+2r+h'] = 1 iff b == 2g+b' and h == h'
    c8 = np.arange(8)
    sqsel = [
        (
            (c8[:, None] // 2 == 2 * g + q[None, :] // 8)
            & (c8[:, None] % 2 == q[None, :] % 2)
        ).astype(np.float32)
        for g in range(2)
    ]
    vals = {
        "ident": np.eye(128, dtype=np.float32),
        "i16": np.eye(16, dtype=np.float32),
        "wmb": np.broadcast_to(w_m[None, :], (128, 128)),
        "wcb": np.broadcast_to(w_c[None, :], (128, 128)),
        "wqb": np.broadcast_to(w_q[None, :], (128, 128)),
        "pairsel": pairsel,
        "hsel": hsel,
        "rsel": (p[:, None] % 4 == np.arange(4)[None, :]).astype(np.float32),
        "b4": (np.arange(4)[:, None] == p[None, :] % 4).astype(np.float32),
        "sqsel0": sqsel[0],
        "sqsel1": sqsel[1],
    }
    packed = np.zeros((128, _CST_W), dtype=np.float32)
    for n, (parts, c0, cl) in _CST_COLS.items():
        packed[:parts, c0 : c0 + cl] = vals[n]
    return {"cstp": packed}


def _run(context, query, w, trace=False):
    from concourse.bass_utils import run_bass_kernel_spmd

    nc = _get_program()
    context = np.ascontiguousarray(context, dtype=np.float32)
    query = np.ascontiguousarray(query, dtype=np.float32)
    consts = _make_const_inputs(w)

    in_maps = []
    for c in range(NCORES):
        m = {
            "ctx": context[c * BPC : (c + 1) * BPC],
            "qry": query[c * BPC : (c + 1) * BPC],
        }
        m.update(consts)
        in_maps.append(m)

    res = run_bass_kernel_spmd(
        nc, in_maps, core_ids=list(range(NCORES)), trace=trace
    )
    out = np.concatenate([res.results[c]["out"] for c in range(NCORES)], axis=0)
    return out, res


def kernel(context, query, c_mask, q_mask, w):
    out, _ = _run(context, query, w, trace=False)
    return out
